# revision 25
# baseline (speedup 1.0000x reference)
"""BiLSTM-CRF forward loss on 8 Trainium2 cores (batch-parallel SPMD).

v2 design — weight-stationary, transposed [units, batch] layout:
- embedding gather -> PE transpose -> xT8 [128, 4estrip, NTOK] fp8 (SBUF)
- input GEMMs and recurrent matmuls use fp8 DoubleRow (contraction 256/instr,
  0.5 cyc/row): stationary lhsT = weight tiles [128, 2, units],
  moving rhs = xT / h state [128, 2, tokens|batch]
- gates land in PSUM as [128 unit, (chunk, batch)]; per-step bias+ih come in
  via two identity matmuls (PSUM preload), so the serial chain is
  MM -> sigmoid -> (f*c, i*g) -> c_new -> tanh -> h  (no transposes, no adds)
- h written directly in transposed layout h1T[d] [128, 4j, T, 8b] fp8, which
  is both the next step's matmul operand and the next layer's GEMM input
- CRF partition in exp domain, 4 independent chains of 2 examples
  (alpha_t+1 = (ETp^T alpha) * exp(emit)), mult on DVE/Pool alternately
- gold path score computed on HOST from the returned logitsT (same logits the
  CRF used, so quantization errors cancel between joint and logZ)
Outputs per core: logitsT [48, NTOK] f32 and logz [1, 8] f32.
"""

import numpy as np

B, T, VOCAB, EMBED, HID, TAGS = 64, 512, 30000, 512, 1024, 48
H1, H2 = HID // 2, HID // 4  # 512, 256
BPC = B // 8  # 8 examples per core
NTOK = T * BPC  # 4096 tokens per core
LN48 = float(np.log(48.0))
GSLOT = [0, 1, 3, 2]  # our gate order (i,f,o,g) -> pytorch row block (i,f,g,o)

_CACHE = {}


def _gate_rows(h):
    """Row permutation: chunk c (=go*nj+jo) of 128 units covers pytorch rows
    pg*h + jo*128 + u, pg = GSLOT[go]. Chunks are gate-major so psum cols
    [i | f | o | g] with unit = 128*jo + p inside each gate block."""
    nj = h // 128
    out = np.empty(4 * h, np.int64)
    for c in range(4 * nj):
        go, jo = c // nj, c % nj
        pg = GSLOT[go]
        out[c * 128:(c + 1) * 128] = pg * h + jo * 128 + np.arange(128)
    return out


def _build_program():
    import concourse.bass as bass
    import concourse.tile as tile
    import concourse.mybir as mybir
    from concourse.vector_clock import ScopedClock, VectorClock
    from concourse.masks import make_identity

    def _patched_drain_and_barrier(self, tick_clock, wait_clock):
        # This container's walrus rejects >2 sem waits on one CTRL
        # instruction; split the kernel-tail drain waits into per-proc
        # NOP waits on the same (in-order) SP queue.
        vc = tick_clock.global_clock
        n = len(vc)
        for p in range(n):
            t = vc[p]
            if t > 0:
                vec = [0] * n
                vec[p] = t
                nop = self.nc.sync.nop()
                wait_clock.add_sem_waits(nop.ins, ScopedClock({None: VectorClock(vec)}))
        self.nc.sync.drain()
        self.nc.all_engine_barrier()
        popped = self.nc._tile_sem_poison_stack.pop()
        assert popped is self._sem_poison
        self.nc.clear_and_free_semaphores(list(self.sems.allocated().values()))
        self.nc.all_engine_barrier()

    tile.TileContext._drain_and_barrier = _patched_drain_and_barrier

    f32 = mybir.dt.float32
    bf16 = mybir.dt.bfloat16
    fp8 = mybir.dt.float8e4
    i32 = mybir.dt.int32
    ACT = mybir.ActivationFunctionType
    ADD = mybir.AluOpType.add
    MULT = mybir.AluOpType.mult
    SUB = mybir.AluOpType.subtract
    DR = mybir.MatmulPerfMode.DoubleRow

    nc = bass.Bass()
    PH = int(__import__("os").environ.get("KPHASES", "99"))

    def din(name, shape, dt=f32):
        return nc.dram_tensor(name, shape, dt, kind="ExternalInput")

    embed_bf = din("embed_bf", [VOCAB, EMBED], bf16)
    seq_tok = din("seq_tok", [NTOK, 1], i32)
    wih1_d = din("wih1", [2, 2, 128, 2, 4 * H1], fp8)
    whh1_d = din("whh1", [2, 2, 128, 2, 4 * H1], fp8)
    biasg1_d = din("biasg1", [2, 128, 128], bf16)
    h01_d = din("h01", [2, 2, 128, 2, BPC], fp8)
    c01_d = din("c01", [2, 128, 32])
    wih2_d = din("wih2", [2, 4, 128, 2, 4 * H2], fp8)
    whh2_d = din("whh2", [2, 128, 2, 4 * H2], fp8)
    biasg2_d = din("biasg2", [2, 128, 64], bf16)
    h02_d = din("h02", [2, 128, 2, BPC], fp8)
    c02_d = din("c02", [2, 128, 16])
    linw_d = din("linw", [2, 128, 2, TAGS], fp8)
    linb_d = din("linb", [TAGS, 1])
    etp_d = din("etp", [TAGS, TAGS], bf16)
    start_d = din("start48", [TAGS, 1])
    ende_d = din("ende", [TAGS, 1], bf16)

    logitsT_d = nc.dram_tensor("logitsT", [TAGS, NTOK], f32, kind="ExternalOutput")
    logz_d = nc.dram_tensor("logz", [1, BPC], f32, kind="ExternalOutput")

    NM = NTOK // 128  # 32 gather chunks (16 timesteps each)

    with tile.TileContext(nc) as tc:
        with tc.tile_pool(name="dram", bufs=1, space="DRAM") as dpool, \
             tc.tile_pool(name="const", bufs=1) as cpool, \
             tc.tile_pool(name="persist", bufs=1) as ppool:

            # L1 pre-activations, (d, m) blocks of [128p, (16t, 16c, 8b)]
            ih1_t = dpool.tile([2, NM, 128, 2048], fp8)  # 16.8 MB
            # L2 pre-activations, (d, m) blocks of [128p, (64t, 8c, 8b)]
            ih2_t = dpool.tile([2, 8, 128, 4096], fp8)  # 8.4 MB

            id128 = cpool.tile([128, 128], bf16)
            make_identity(nc, id128[:])

            wih1sb, whh1sb, h01sb = {}, {}, {}
            for d in range(2):
                for kk in range(2):
                    w = cpool.tile([128, 2, 4 * H1], fp8, tag=f"wih1_{d}{kk}",
                                   name=f"wih1_{d}{kk}")
                    nc.sync.dma_start(w[:], wih1_d[d, kk])
                    wih1sb[(d, kk)] = w
                    w = cpool.tile([128, 2, 4 * H1], fp8, tag=f"whh1_{d}{kk}",
                                   name=f"whh1_{d}{kk}")
                    nc.sync.dma_start(w[:], whh1_d[d, kk])
                    whh1sb[(d, kk)] = w
                    h = cpool.tile([128, 2, BPC], fp8, tag=f"h01_{d}{kk}",
                                   name=f"h01_{d}{kk}")
                    nc.sync.dma_start(h[:], h01_d[d, kk])
                    h01sb[(d, kk)] = h
            wih2sb, biasg = {}, {}
            for d in range(2):
                for kk in range(4):
                    w = cpool.tile([128, 2, 4 * H2], fp8, tag=f"wih2_{d}{kk}",
                                   name=f"wih2_{d}{kk}")
                    nc.sync.dma_start(w[:], wih2_d[d, kk])
                    wih2sb[(d, kk)] = w
            whh2sb, h02sb = {}, {}
            for d in range(2):
                w = cpool.tile([128, 2, 4 * H2], fp8, tag=f"whh2_{d}", name=f"whh2_{d}")
                nc.sync.dma_start(w[:], whh2_d[d])
                whh2sb[d] = w
                h = cpool.tile([128, 2, BPC], fp8, tag=f"h02_{d}", name=f"h02_{d}")
                nc.sync.dma_start(h[:], h02_d[d])
                h02sb[d] = h
                bgl = cpool.tile([128, 128], bf16, tag=f"bg1_{d}", name=f"bg1_{d}")
                nc.sync.dma_start(bgl[:], biasg1_d[d])
                biasg[(1, d)] = bgl
                bgl = cpool.tile([128, 64], bf16, tag=f"bg2_{d}", name=f"bg2_{d}")
                nc.sync.dma_start(bgl[:], biasg2_d[d])
                biasg[(2, d)] = bgl
            linwsb = []
            for kk in range(2):
                w = cpool.tile([128, 2, TAGS], fp8, tag=f"linw_{kk}", name=f"linw_{kk}")
                nc.sync.dma_start(w[:], linw_d[kk])
                linwsb.append(w)
            linb_sb = cpool.tile([TAGS, 1], f32)
            nc.sync.dma_start(linb_sb[:], linb_d[:])
            etp_sb = cpool.tile([TAGS, TAGS], bf16)
            nc.sync.dma_start(etp_sb[:], etp_d[:])
            start_sb = cpool.tile([TAGS, 1], f32)
            nc.sync.dma_start(start_sb[:], start_d[:])
            ende_sb = cpool.tile([TAGS, 1], bf16)
            nc.sync.dma_start(ende_sb[:], ende_d[:])

            # persistent transposed activations
            h1T = [ppool.tile([128, 4, T, BPC], fp8, tag=f"h1T{d}", name=f"h1T{d}")
                   for d in range(2)]
            h2T = [ppool.tile([128, 2, T, BPC], fp8, tag=f"h2T{d}", name=f"h2T{d}")
                   for d in range(2)]

            # ====== P1+P2: embedding gather/transpose + L1 input GEMM ======
            with tc.tile_pool(name="px", bufs=1) as xpool, \
                 tc.tile_pool(name="p1", bufs=3) as sp, \
                 tc.tile_pool(name="p1s", bufs=3) as stp, \
                 tc.tile_pool(name="p1t", bufs=4, space="PSUM") as pst, \
                 tc.tile_pool(name="p1p", bufs=4, space="PSUM") as psp:
                xT8 = xpool.tile([128, 4, NTOK], fp8)
                for m in range(NM if PH >= 1 else 0):
                    idx = sp.tile([128, 1], i32, tag="idx")
                    nc.sync.dma_start(idx[:], seq_tok[128 * m:128 * (m + 1), :])
                    xg = sp.tile([128, EMBED], bf16, tag="xg")
                    nc.gpsimd.indirect_dma_start(
                        out=xg[:], out_offset=None, in_=embed_bf[:],
                        in_offset=bass.IndirectOffsetOnAxis(ap=idx[:, :1], axis=0))
                    for e in range(4):
                        pt = pst.tile([128, 128], bf16, space="PSUM", tag="pt")
                        nc.tensor.transpose(out=pt[:], in_=xg[:, 128 * e:128 * (e + 1)],
                                            identity=id128[:])
                        nc.vector.tensor_copy(xT8[:, e, 128 * m:128 * (m + 1)], pt[:])
                    if PH < 2:
                        continue
                    for d in range(2):
                        stg = stp.tile([128, 16, 16, BPC], fp8, tag=f"stg{d}")
                        for g in range(4):
                            pg4 = psp.tile([128, 4, 16, BPC], f32, space="PSUM",
                                           tag="pg2")
                            for cc in range(4):
                                c = 4 * g + cc
                                for kk in range(2):
                                    nc.tensor.matmul(
                                        pg4[:, cc, :, :],
                                        lhsT=wih1sb[(d, kk)][:, :, 128 * c:128 * (c + 1)],
                                        rhs=xT8[:, 2 * kk:2 * kk + 2,
                                                128 * m:128 * (m + 1)],
                                        start=(kk == 0), stop=(kk == 1),
                                        perf_mode=DR, skip_group_check=True)
                            src = pg4[:].rearrange("p c t b -> p t c b")
                            dst = stg[:, :, 4 * g:4 * (g + 1), :]
                            if g % 2 == 0:
                                nc.vector.tensor_copy(dst, src)
                            else:
                                nc.scalar.copy(dst, src)
                        nc.sync.dma_start(ih1_t[d, m], stg[:])

            # ========= P3: L1 scans (fwd + bwd) + interleaved P4 ===========
            with tc.tile_pool(name="st3", bufs=1) as stp, \
                 tc.tile_pool(name="ihp", bufs=3) as ihp, \
                 tc.tile_pool(name="p3", bufs=4) as sp, \
                 tc.tile_pool(name="p4s", bufs=3) as stp4, \
                 tc.tile_pool(name="p4p", bufs=2, space="PSUM") as psp4, \
                 tc.tile_pool(name="p3g", bufs=2, space="PSUM") as psg:

                def emit_p4(m):
                    # L2 input GEMM for token block m (both dirs); emitted as
                    # soon as h1T covers t in [64m, 64m+64) so it fills P3's
                    # engine-idle slack
                    for d in range(2):
                        stg = stp4.tile([128, 64, 8, BPC], fp8, tag="stg4",
                                        name=f"stg4_{d}_{m}")
                        for c in range(8):
                            pg = psp4.tile([128, 512], f32, space="PSUM", tag="pg4",
                                           name=f"pg4_{d}_{m}_{c}")
                            for kk in range(4):
                                rh = h1T[kk // 2][:, 2 * (kk % 2):2 * (kk % 2) + 2,
                                                  64 * m:64 * (m + 1), :]
                                nc.tensor.matmul(
                                    pg[:],
                                    lhsT=wih2sb[(d, kk)][:, :, 128 * c:128 * (c + 1)],
                                    rhs=rh, start=(kk == 0), stop=(kk == 3),
                                    perf_mode=DR, skip_group_check=True)
                            dst = stg[:, :, c, :]
                            src = pg[:].rearrange("p (t b) -> p t b", b=BPC)
                            if (c + m) % 2 == 0:
                                nc.vector.tensor_copy(dst, src)
                            else:
                                nc.scalar.copy(dst, src)
                        nc.sync.dma_start(ih2_t[d, m], stg[:])

                p4_at = {}
                if PH >= 4:
                    for m in range(8):
                        p4_at.setdefault(max(64 * m + 63, T - 1 - 64 * m),
                                         []).append(m)

                c1S = {}
                for d in range(2):
                    for par in range(2):
                        c1S[(d, par)] = stp.tile([128, 32], f32, tag=f"c1_{d}{par}",
                                                 name=f"c1_{d}{par}")
                    nc.sync.dma_start(c1S[(d, 0)][:], c01_d[d])

                ihm = {0: {}, 1: {}}

                def prefetch1(d, mb):
                    tl = ihp.tile([128, 16, 16, BPC], fp8, tag=f"ihm{d}",
                                  name=f"ihm{d}_{mb}")
                    nc.sync.dma_start(tl[:], ih1_t[d, mb])
                    ihm[d][mb] = tl

                if PH >= 3:
                    prefetch1(0, 0)
                    prefetch1(1, NM - 1)
                    prefetch1(0, 1)
                    prefetch1(1, NM - 2)
                for s in range(T if PH >= 3 else 0):
                    if s % 16 == 0 and s > 0:
                        mbf, mbb = s // 16 + 1, NM - 2 - s // 16
                        if mbf < NM:
                            prefetch1(0, mbf)
                        if mbb >= 0:
                            prefetch1(1, mbb)
                    # stage-interleaved across the two direction chains so the
                    # in-order engine queues advance both in lockstep
                    pg, sig, tg, t1, t2, th = {}, {}, {}, {}, {}, {}
                    for d in range(2):
                        t = s if d == 0 else T - 1 - s
                        mb, ti = t // 16, t % 16
                        pg[d] = psg.tile([128, 128], f32, space="PSUM", tag=f"pg{d}",
                                         name=f"pg{d}_{s}")
                        nc.tensor.matmul(pg[d][:], lhsT=id128[:],
                                         rhs=ihm[d][mb][:, ti, :, :],
                                         start=True, stop=False, skip_group_check=True)
                        nc.tensor.matmul(pg[d][:], lhsT=id128[:], rhs=biasg[(1, d)][:],
                                         start=False, stop=False, skip_group_check=True)
                        for c in range(16):
                            for kk in range(2):
                                if s == 0:
                                    rh = h01sb[(d, kk)][:]
                                else:
                                    tp = (s - 1) if d == 0 else (T - s)
                                    rh = h1T[d][:, 2 * kk:2 * kk + 2, tp, :]
                                nc.tensor.matmul(
                                    pg[d][:, 8 * c:8 * (c + 1)],
                                    lhsT=whh1sb[(d, kk)][:, :, 128 * c:128 * (c + 1)],
                                    rhs=rh, start=False,
                                    stop=(c == 15 and kk == 1),
                                    perf_mode=DR, skip_group_check=True)
                    for d in range(2):
                        sig[d] = sp.tile([128, 128], bf16, tag=f"sig{d}",
                                         name=f"sig{d}_{s}")
                        nc.scalar.activation(sig[d][:], pg[d][:], ACT.Sigmoid)
                    for d in range(2):
                        t1[d] = sp.tile([128, 32], f32, tag=f"t1_{d}",
                                        name=f"t1_{d}_{s}")
                        nc.vector.tensor_tensor(out=t1[d][:], in0=sig[d][:, 32:64],
                                                in1=c1S[(d, s % 2)][:], op=MULT)
                        t2[d] = sp.tile([128, 32], f32, tag=f"t2_{d}",
                                        name=f"t2_{d}_{s}")
                        nc.vector.tensor_tensor(out=t2[d][:], in0=sig[d][:, 0:32],
                                                in1=sig[d][:, 96:128], op=MULT)
                    for d in range(2):
                        nc.vector.scalar_tensor_tensor(
                            out=c1S[(d, (s + 1) % 2)][:], in0=t2[d][:], scalar=2.0,
                            in1=t1[d][:], op0=MULT, op1=ADD)
                        nc.vector.tensor_tensor(out=c1S[(d, (s + 1) % 2)][:],
                                                in0=c1S[(d, (s + 1) % 2)][:],
                                                in1=sig[d][:, 0:32], op=SUB)
                    for d in range(2):
                        th[d] = sp.tile([128, 32], bf16, tag=f"th{d}",
                                        name=f"th{d}_{s}")
                        nc.scalar.activation(th[d][:], c1S[(d, (s + 1) % 2)][:],
                                             ACT.Tanh)
                    for d in range(2):
                        t = s if d == 0 else T - 1 - s
                        nc.vector.tensor_tensor(
                            out=h1T[d][:, :, t, :],
                            in0=sig[d][:, 64:96].rearrange("p (j b) -> p j b", j=4),
                            in1=th[d][:].rearrange("p (j b) -> p j b", j=4), op=MULT)
                    for m in p4_at.get(s, []):
                        emit_p4(m)

            # ================= P5: L2 scans ================================
            with tc.tile_pool(name="st5", bufs=1) as stp, \
                 tc.tile_pool(name="ihp5", bufs=2) as ihp5, \
                 tc.tile_pool(name="p5", bufs=4) as sp, \
                 tc.tile_pool(name="p5g", bufs=2, space="PSUM") as psg:
                c2S = {}
                for d in range(2):
                    for par in range(2):
                        c2S[(d, par)] = stp.tile([128, 16], f32, tag=f"c2_{d}{par}",
                                                 name=f"c2_{d}{par}")
                    nc.sync.dma_start(c2S[(d, 0)][:], c02_d[d])

                ihm2 = {0: {}, 1: {}}

                def prefetch2(d, mb):
                    tl = ihp5.tile([128, 64, 8, BPC], fp8, tag=f"ihm2_{d}",
                                   name=f"ihm2_{d}_{mb}")
                    nc.sync.dma_start(tl[:], ih2_t[d, mb])
                    ihm2[d][mb] = tl

                if PH >= 5:
                    prefetch2(0, 0)
                    prefetch2(1, 7)
                for s in range(T if PH >= 5 else 0):
                    if s % 64 == 0 and s + 64 < T:
                        prefetch2(0, s // 64 + 1)
                        prefetch2(1, 6 - s // 64)
                    pg, sig, tg, t1, t2, th = {}, {}, {}, {}, {}, {}
                    for d in range(2):
                        t = s if d == 0 else T - 1 - s
                        pg[d] = psg.tile([128, 64], f32, space="PSUM", tag=f"pg5{d}",
                                         name=f"pg5{d}_{s}")
                        nc.tensor.matmul(pg[d][:], lhsT=id128[:],
                                         rhs=ihm2[d][t // 64][:, t % 64, :, :],
                                         start=True, stop=False, skip_group_check=True)
                        nc.tensor.matmul(pg[d][:], lhsT=id128[:], rhs=biasg[(2, d)][:],
                                         start=False, stop=False, skip_group_check=True)
                        for c in range(8):
                            if s == 0:
                                rh = h02sb[d][:]
                            else:
                                tp = (s - 1) if d == 0 else (T - s)
                                rh = h2T[d][:, :, tp, :]
                            nc.tensor.matmul(
                                pg[d][:, 8 * c:8 * (c + 1)],
                                lhsT=whh2sb[d][:, :, 128 * c:128 * (c + 1)],
                                rhs=rh, start=False, stop=(c == 7),
                                perf_mode=DR, skip_group_check=True)
                    for d in range(2):
                        sig[d] = sp.tile([128, 64], bf16, tag=f"sg5{d}",
                                         name=f"sg5{d}_{s}")
                        nc.scalar.activation(sig[d][:], pg[d][:], ACT.Sigmoid)
                    for d in range(2):
                        t1[d] = sp.tile([128, 16], f32, tag=f"t15_{d}",
                                        name=f"t15_{d}_{s}")
                        nc.vector.tensor_tensor(out=t1[d][:], in0=sig[d][:, 16:32],
                                                in1=c2S[(d, s % 2)][:], op=MULT)
                        t2[d] = sp.tile([128, 16], f32, tag=f"t25_{d}",
                                        name=f"t25_{d}_{s}")
                        nc.vector.tensor_tensor(out=t2[d][:], in0=sig[d][:, 0:16],
                                                in1=sig[d][:, 48:64], op=MULT)
                    for d in range(2):
                        nc.vector.scalar_tensor_tensor(
                            out=c2S[(d, (s + 1) % 2)][:], in0=t2[d][:], scalar=2.0,
                            in1=t1[d][:], op0=MULT, op1=ADD)
                        nc.vector.tensor_tensor(out=c2S[(d, (s + 1) % 2)][:],
                                                in0=c2S[(d, (s + 1) % 2)][:],
                                                in1=sig[d][:, 0:16], op=SUB)
                    for d in range(2):
                        th[d] = sp.tile([128, 16], bf16, tag=f"th5{d}",
                                        name=f"th5{d}_{s}")
                        nc.scalar.activation(th[d][:], c2S[(d, (s + 1) % 2)][:],
                                             ACT.Tanh)
                    for d in range(2):
                        t = s if d == 0 else T - 1 - s
                        nc.vector.tensor_tensor(
                            out=h2T[d][:, :, t, :],
                            in0=sig[d][:, 32:48].rearrange("p (j b) -> p j b", j=2),
                            in1=th[d][:].rearrange("p (j b) -> p j b", j=2), op=MULT)

            # ================= P6: linear -> logitsT + Esb =================
            logitsT_sb = ppool.tile([TAGS, NTOK], f32, tag="logitsT_sb")
            Esb = ppool.tile([TAGS, NTOK], f32, tag="Esb")
            with tc.tile_pool(name="p6p", bufs=2, space="PSUM") as psp:
                for m in range(8 if PH >= 6 else 0):
                    pl = psp.tile([TAGS, 512], f32, space="PSUM", tag="pl")
                    for kk in range(2):
                        nc.tensor.matmul(pl[:], lhsT=linwsb[kk][:],
                                         rhs=h2T[kk][:, :, 64 * m:64 * (m + 1), :],
                                         start=(kk == 0), stop=(kk == 1),
                                         perf_mode=DR, skip_group_check=True)
                    nc.scalar.activation(logitsT_sb[:, 512 * m:512 * (m + 1)], pl[:],
                                         ACT.Identity, bias=linb_sb[:, 0:1])
                    nc.scalar.activation(Esb[:, 512 * m:512 * (m + 1)], pl[:],
                                         ACT.Exp, bias=linb_sb[:, 0:1])
                nc.sync.dma_start(logitsT_d[:], logitsT_sb[:])

            # ================= P7: CRF partition (exp domain) ==============
            # 4 independent chains: 2 on DVE (3 examples each) and 2 on Act
            # (1 example each, emit-multiply via activation scale)
            CHAINS = [(0, 3, "dve"), (3, 6, "dve"), (6, 7, "act"), (7, 8, "act")]
            with tc.tile_pool(name="p7", bufs=1) as sp, \
                 tc.tile_pool(name="p7a", bufs=3) as ap7, \
                 tc.tile_pool(name="p7p2", bufs=2, space="PSUM") as psp2, \
                 tc.tile_pool(name="p7p", bufs=1, space="PSUM") as psp:
                alpha = {}
                for ch, (b0, b1, eng) in enumerate(CHAINS if PH >= 7 else []):
                    a0 = ap7.tile([TAGS, b1 - b0], bf16, tag=f"al{ch}",
                                  name=f"al{ch}_0")
                    nc.scalar.activation(a0[:], logitsT_sb[:, b0:b1],
                                         ACT.Exp, bias=start_sb[:, 0:1])
                    alpha[ch] = a0
                for t in range(1, T if PH >= 7 else 1):
                    for ch, (b0, b1, eng) in enumerate(CHAINS):
                        pool = psp2 if eng == "dve" else psp
                        pm = pool.tile([TAGS, b1 - b0], f32, space="PSUM",
                                       tag=f"pm{ch}", name=f"pm{ch}_{t}")
                        nc.tensor.matmul(pm[:], lhsT=etp_sb[:], rhs=alpha[ch][:],
                                         start=True, stop=True)
                        a = ap7.tile([TAGS, b1 - b0], bf16, tag=f"al{ch}",
                                     name=f"al{ch}_{t}")
                        if eng == "dve":
                            nc.vector.tensor_tensor(
                                out=a[:], in0=pm[:],
                                in1=Esb[:, BPC * t + b0:BPC * t + b1], op=MULT)
                        else:
                            nc.scalar.activation(
                                a[:], pm[:], ACT.Identity,
                                scale=Esb[:, BPC * t + b0:BPC * t + b1])
                        alpha[ch] = a
                logz_sb = sp.tile([1, BPC], f32, tag="logz_sb")
                if PH >= 7:
                    for ch, (b0, b1, eng) in enumerate(CHAINS):
                        pf = psp.tile([1, b1 - b0], f32, space="PSUM", tag="pf",
                                      name=f"pf{ch}")
                        nc.tensor.matmul(pf[:], lhsT=ende_sb[:], rhs=alpha[ch][:],
                                         start=True, stop=True)
                        nc.scalar.activation(logz_sb[:, b0:b1], pf[:], ACT.Ln)
                    nc.vector.tensor_scalar_add(logz_sb[:], logz_sb[:],
                                                float((T - 1) * LN48))
                else:
                    nc.gpsimd.memset(logz_sb[:], 0.0)
                nc.sync.dma_start(logz_d[:], logz_sb[:])

    _split_waits(nc, maxw=int(__import__("os").environ.get("KMAXW", "1")))
    return nc


def _split_waits(nc, maxw=2):
    """This container's walrus rejects instructions carrying more than a
    couple of semaphore waits. Hoist extras onto preceding same-engine
    NoOps (engines execute their stream in order, so this preserves the
    happens-before)."""
    import concourse.mybir as mybir
    import bass_rust
    compute_ops = {"Matmult", "Activation", "TensorTensor", "TensorScalar",
                   "TensorCopy", "TensorReduce", "Memset", "Iota",
                   "AffineSelect", "TensorTensorScan", "Select"}
    n_added = 0
    for fn in nc.m.functions:
        for blk in fn.blocks:
            insts = list(blk.instructions)
            out = []
            dirty = False
            for inst in insts:
                mw = 2 if (maxw == 0 and str(inst.opcode) in compute_ops) else max(1, maxw)
                si = inst.sync_info
                if si is not None and len(si.on_wait) > mw:
                    waits = list(si.on_wait)
                    extra, keep = waits[:-mw], waits[-mw:]
                    for i in range(0, len(extra), mw):
                        nop = mybir.InstNoOp(
                            name=f"{inst.name}_hw{i}", ins=[], outs=[])
                        nop.engine = inst.engine
                        nop.sync_info = bass_rust.SyncInfo(
                            on_wait=extra[i:i + mw], on_update=[])
                        out.append(nop)
                        n_added += 1
                    inst.sync_info = bass_rust.SyncInfo(
                        on_wait=keep, on_update=list(si.on_update))
                    dirty = True
                out.append(inst)
            if dirty:
                blk.instructions = out
    return n_added


def _prep_inputs(inputs):
    import ml_dtypes
    bf = ml_dtypes.bfloat16
    e4 = ml_dtypes.float8_e4m3fn

    g = {k: np.asarray(v) for k, v in inputs.items()}
    seq = g["sequences"].astype(np.int64)

    gr1 = _gate_rows(H1)  # 2048
    gr2 = _gate_rows(H2)  # 1024

    shared = {}
    shared["embed_bf"] = np.ascontiguousarray(g["embed_table"].astype(bf))
    # tanh(x) = 2*sigmoid(2x) - 1: pre-double the g-gate (pytorch block 2)
    # rows of W_ih/W_hh/bias so one Sigmoid instruction covers all gates
    def dbl(W, h):
        W = W.astype(np.float32).copy()
        W[2 * h:3 * h] *= 2.0
        return W

    wih1 = np.zeros((2, 2, 128, 2, 4 * H1), e4)
    whh1 = np.zeros((2, 2, 128, 2, 4 * H1), e4)
    biasg1 = np.zeros((2, 128, 128), bf)
    for d, sfx in enumerate(["1f", "1b"]):
        Wg = dbl(g["w_ih" + sfx], H1)[gr1]  # [2048, 512]
        wih1[d] = Wg.T.reshape(2, 2, 128, 4 * H1).transpose(0, 2, 1, 3).astype(e4)
        Wh = dbl(g["w_hh" + sfx], H1)[gr1]  # [2048, 512]
        whh1[d] = Wh.T.reshape(2, 2, 128, 4 * H1).transpose(0, 2, 1, 3).astype(e4)
        bb = dbl((g["b_ih" + sfx] + g["b_hh" + sfx])[:, None], H1)[gr1, 0]
        biasg1[d] = np.repeat(bb.reshape(16, 128).T[:, :, None], BPC,
                              axis=2).reshape(128, 128).astype(bf)
    wih2 = np.zeros((2, 4, 128, 2, 4 * H2), e4)
    whh2 = np.zeros((2, 128, 2, 4 * H2), e4)
    biasg2 = np.zeros((2, 128, 64), bf)
    for d, sfx in enumerate(["2f", "2b"]):
        Wg = dbl(g["w_ih" + sfx], H2)[gr2]  # [1024, 1024]
        wih2[d] = Wg.T.reshape(4, 2, 128, 4 * H2).transpose(0, 2, 1, 3).astype(e4)
        Wh = dbl(g["w_hh" + sfx], H2)[gr2]  # [1024, 256]
        whh2[d] = Wh.T.reshape(2, 128, 4 * H2).transpose(1, 0, 2).astype(e4)
        bb = dbl((g["b_ih" + sfx] + g["b_hh" + sfx])[:, None], H2)[gr2, 0]
        biasg2[d] = np.repeat(bb.reshape(8, 128).T[:, :, None], BPC,
                              axis=2).reshape(128, 64).astype(bf)
    shared["wih1"], shared["whh1"], shared["biasg1"] = wih1, whh1, biasg1
    shared["wih2"], shared["whh2"], shared["biasg2"] = wih2, whh2, biasg2
    lw = g["lin_w"].astype(np.float32)  # [48, 512]
    shared["linw"] = lw.T.reshape(2, 2, 128, TAGS).transpose(0, 2, 1, 3).astype(e4)
    shared["linb"] = g["lin_b"].astype(np.float32)[:, None]
    shared["etp"] = np.exp(g["transitions"].astype(np.float64) - LN48).astype(bf)
    shared["start48"] = g["start_trans"].astype(np.float32)[:, None]
    shared["ende"] = np.exp(g["end_trans"].astype(np.float64)).astype(bf)[:, None]

    in_maps = []
    for core in range(8):
        sl = slice(core * BPC, (core + 1) * BPC)
        m = dict(shared)
        m["seq_tok"] = seq[sl].T.reshape(NTOK).astype(np.int32)[:, None]
        h0 = g["h0"][:, sl].astype(np.float32)  # [2, 8, 512]
        m["h01"] = h0.reshape(2, BPC, 2, 2, 128).transpose(0, 2, 4, 3, 1).astype(e4)
        c0 = g["c0"][:, sl].astype(np.float32)
        m["c01"] = np.ascontiguousarray(
            c0.reshape(2, BPC, 4, 128).transpose(0, 3, 2, 1).reshape(2, 128, 32))
        h1 = g["h1"][:, sl].astype(np.float32)  # [2, 8, 256]
        m["h02"] = h1.reshape(2, BPC, 2, 128).transpose(0, 3, 2, 1).astype(e4)
        c1 = g["c1"][:, sl].astype(np.float32)
        m["c02"] = np.ascontiguousarray(
            c1.reshape(2, BPC, 2, 128).transpose(0, 3, 2, 1).reshape(2, 128, 16))
        in_maps.append(m)
    return in_maps


def kernel(**inputs) -> np.ndarray:
    import time
    from concourse.bass_utils import run_bass_kernel_spmd

    if "nc" not in _CACHE:
        _CACHE["nc"] = _build_program()
    nc = _CACHE["nc"]

    in_maps = _prep_inputs(inputs)
    res = None
    for attempt in range(3):
        try:
            res = run_bass_kernel_spmd(nc, in_maps, core_ids=list(range(8)))
            break
        except Exception:
            # transient NRT_EXEC_UNIT_UNRECOVERABLE after wedged runs —
            # observed to recover after ~60s
            if attempt == 2:
                raise
            time.sleep(60)

    tags = np.asarray(inputs["tags"]).astype(np.int64)
    mask = np.asarray(inputs["mask"]).astype(bool)
    trans = np.asarray(inputs["transitions"]).astype(np.float64)
    start = np.asarray(inputs["start_trans"]).astype(np.float64)
    end = np.asarray(inputs["end_trans"]).astype(np.float64)

    loss = np.float64(0.0)
    bidx = np.arange(BPC)
    for core, r in enumerate(res.results):
        lg = r["logitsT"].astype(np.float64)  # [48, NTOK]
        logz = r["logz"].astype(np.float64)[0]  # [8]
        sl = slice(core * BPC, (core + 1) * BPC)
        tt = tags[sl].T  # [T, 8]
        mt = mask[sl].T
        mf = mt.astype(np.float64)
        ltb = lg.reshape(TAGS, T, BPC).transpose(1, 2, 0)  # [T, 8, 48]
        emit = np.take_along_axis(ltb, tt[..., None], axis=-1)[..., 0]  # [T, 8]
        score = start[tt[0]]
        score = score + np.sum(trans[tt[:-1], tt[1:]] * mf[1:], axis=0)
        score = score + np.sum(emit[:-1] * mf[:-1], axis=0)
        last_idx = np.sum(mt.astype(np.int64), axis=0) - 1
        last_tags = tt[last_idx, bidx]
        score = score + end[last_tags] + ltb[-1, bidx, last_tags] * mf[-1]
        loss += np.sum(score - logz)
    return np.float32(-loss)
